# revision 11
# baseline (speedup 1.0000x reference)
"""Contextual attention kernel for Trainium2 (8 NeuronCores, SPMD).

Sharding: 8 cores = 2 batches x 4 query-row-blocks (1024 queries each).
Per core the device computes, for its query block:
  scores  S[q, l]   = (sum_ck Q[ck,q] * Kraw[ck,l]) * s[l]   (fp32r matmuls)
          where s[l] = SOFTMAX_SCALE * mm[l] / max(norm[l], 1e-4)
  softmax over l (max via DVE reduce, exp+sum fused on ACT)
  argmax over l (DVE max8/max_index on masked normalized probs)
  probs   P2 = exp(S - max) * mm / (4*den)
  recon   out[kikj, c, q] = sum_l raw_w[l, kikj, c] * P2[l, q]  (fp16 matmuls)
Host does: downsample/pad/pack layout prep, per-l norm+mask scales (tiny),
overlap-add of the 4x4 transposed-conv patches, offset arithmetic.
"""
import numpy as np

import concourse.bass as bass
import concourse.mybir as mybir
from concourse import bacc
from concourse.tile import TileContext
from concourse.masks import make_identity
from concourse.bass_utils import run_bass_kernel_spmd

F32 = mybir.dt.float32
F32R = mybir.dt.float32r
F16 = mybir.dt.float16
U32 = mybir.dt.uint32

OFFS = [(dp, dq) for dp in range(3) for dq in range(3)]

B, C, H, W = 2, 128, 128, 128
HF = WF = 64          # downsampled dims
L = HF * WF           # 4096 patches
QPC = 1024            # queries per core (16 h-rows x 64)
N_SUB = 4             # subs per core, 256 queries each
SCALE = 10.0

_CACHE = {}


def _build(score_mode="f32r", sbufs=4, pbufs=2, split_in=True, rwbufs=64, ptchunks=8):
    sdt = {"f32r": F32R, "f32": F32}[score_mode]
    nc = bacc.Bacc("TRN2", target_bir_lowering=False)

    fsq_d = nc.dram_tensor("fsq", [9, 128, QPC], sdt, kind="ExternalInput")
    bsb_d = nc.dram_tensor("bsb", [128, 66, 66], sdt, kind="ExternalInput")
    sbc_d = nc.dram_tensor("sbc", [128, L], F32, kind="ExternalInput")
    mmb_d = nc.dram_tensor("mmb", [128, L], F32, kind="ExternalInput")
    btp_d = nc.dram_tensor("btp", [130, 130, 128], F16, kind="ExternalInput")

    rec_d = nc.dram_tensor("recon", [16, 128, QPC], F32, kind="ExternalOutput")
    idx_d = nc.dram_tensor("idxo", [8, 128, 8], U32, kind="ExternalOutput")
    top_d = nc.dram_tensor("tops", [8, 128, 8], F32, kind="ExternalOutput")

    with TileContext(nc) as tc:
        with tc.tile_pool(name="ps", bufs=8, space="PSUM") as psp, \
             tc.tile_pool(name="drp", bufs=1, space="DRAM") as drp, \
             tc.tile_pool(name="smp", bufs=24) as smp:
            PTd = drp.tile([32, 128, QPC], F16)

            with tc.tile_pool(name="stat", bufs=1) as st, \
                 tc.tile_pool(name="Sp", bufs=sbufs) as Sp, \
                 tc.tile_pool(name="Pp", bufs=pbufs) as Pp, \
                 tc.tile_pool(name="ptp", bufs=4) as ptp:
                fsq = st.tile([128, 9, QPC], sdt)
                bsb = st.tile([128, 66, 66], sdt)
                if split_in:
                    for t in range(9):
                        nc.sync.dma_start(fsq[:, t, :], fsq_d[t])
                    for lb in range(8):
                        r0 = lb * 8
                        r1 = min(66, r0 + 10)
                        nc.sync.dma_start(bsb[:, r0:r1, :], bsb_d[:, r0:r1, :])
                else:
                    nc.sync.dma_start(fsq,
                                      fsq_d.ap().rearrange("t c q -> c t q"))
                    nc.sync.dma_start(bsb, bsb_d[:])
                ssb = st.tile([128, L], F32)
                nc.sync.dma_start(ssb, sbc_d[:])
                mmb = st.tile([128, L], F32)
                nc.sync.dma_start(mmb, mmb_d[:])
                ident = st.tile([128, 128], F32)
                make_identity(nc, ident[:])

                def phase_a(sub):
                    Ss = [Sp.tile([128, L], F32, name=f"S_{sub}_{i}", tag="S")
                          for i in range(2)]
                    for lb in range(8):
                        for qt2 in range(2):
                            qt = sub * 2 + qt2
                            ps = psp.tile([128, 512], F32,
                                          name=f"ps_{qt}_{lb}", tag="ps")
                            for t, (dp, dq) in enumerate(OFFS):
                                lhsT = fsq[:, t, qt * 128:(qt + 1) * 128]
                                rhs = bsb[:, lb * 8 + dp:lb * 8 + dp + 8,
                                          dq:dq + 64]
                                nc.tensor.matmul(ps[:], lhsT, rhs,
                                                 start=(t == 0), stop=(t == 8))
                            nc.vector.tensor_tensor(
                                Ss[qt2][:, lb * 512:(lb + 1) * 512], ps[:],
                                ssb[:, lb * 512:(lb + 1) * 512],
                                op=mybir.AluOpType.mult)
                    return Ss

                def phase_b(sub, Ss):
                    for qt2 in range(2):
                        qt = sub * 2 + qt2
                        S = Ss[qt2]
                        negmax = smp.tile([128, 1], F32, name=f"ngm_{qt}",
                                          tag="negmax")
                        nc.vector.tensor_reduce(negmax[:], S[:],
                                                axis=mybir.AxisListType.X,
                                                op=mybir.AluOpType.max,
                                                negate=True)
                        P = Pp.tile([128, L], F32, name=f"P_{qt}", tag="P")
                        den = smp.tile([128, 1], F32, name=f"den_{qt}",
                                       tag="den")
                        nc.scalar.activation(P[:], S[:],
                                             mybir.ActivationFunctionType.Exp,
                                             bias=negmax[:, 0:1], scale=1.0,
                                             accum_out=den[:, 0:1])
                        den4 = smp.tile([128, 1], F32, name=f"den4_{qt}",
                                        tag="den4")
                        nc.vector.tensor_scalar_mul(den4[:], den[:], 4.0)
                        inv4 = smp.tile([128, 1], F32, name=f"inv4_{qt}",
                                        tag="inv4")
                        nc.vector.reciprocal(inv4[:], den4[:])
                        # P = (P * inv4) * mm   (masked, normalized, /4)
                        nc.vector.scalar_tensor_tensor(
                            P[:], P[:], inv4[:, 0:1], mmb[:],
                            op0=mybir.AluOpType.mult,
                            op1=mybir.AluOpType.mult)
                        top8 = smp.tile([128, 8], F32, name=f"top8_{qt}",
                                        tag="top8")
                        idx8 = smp.tile([128, 8], U32, name=f"idx8_{qt}",
                                        tag="idx8")
                        nc.vector.max(top8[:], P[:])
                        nc.vector.max_index(idx8[:], top8[:], P[:])
                        nc.sync.dma_start(idx_d[qt], idx8[:])
                        nc.sync.dma_start(top_d[qt], top8[:])
                        for g in range(8):
                            pst = psp.tile([128, 4, 128], F32,
                                           name=f"pst_{qt}_{g}", tag="ps")
                            for j in range(4):
                                lt = g * 4 + j
                                nc.tensor.transpose(
                                    pst[:, j, :],
                                    P[:, lt * 128:(lt + 1) * 128], ident[:])
                            ptsb = ptp.tile([128, 4, 128], F16,
                                            name=f"ptsb_{qt}_{g}", tag="ptsb")
                            nc.vector.tensor_copy(ptsb[:], pst[:])
                            dst = PTd[g * 4:(g + 1) * 4, :,
                                      qt * 128:(qt + 1) * 128]
                            nc.sync.dma_start(
                                dst.rearrange("a p q -> p a q"), ptsb[:])

                # software pipeline: scores(sub+1) issue before softmax(sub)
                # so the PE never waits on the softmax chain
                prev = None
                for sub in range(N_SUB):
                    cur = (sub, phase_a(sub))
                    if prev is not None:
                        phase_b(*prev)
                    prev = cur
                phase_b(*prev)

            # phase C: reconstruction
            with tc.tile_pool(name="PTp", bufs=1) as PTp, \
                 tc.tile_pool(name="rwp", bufs=rwbufs) as rwp, \
                 tc.tile_pool(name="osp", bufs=4) as osp:
                PT = PTp.tile([128, 32, QPC], F16)
                step = 32 // ptchunks
                for c4 in range(ptchunks):
                    lo, hi = c4 * step, (c4 + 1) * step
                    src = PTd[lo:hi, :, :]
                    nc.sync.dma_start(PT[:, lo:hi, :],
                                      src.rearrange("a p q -> p a q"))
                for ki in range(4):
                    rws = []
                    for lt in range(32):
                        rw = rwp.tile([128, 4, 128], F16,
                                      name=f"rw_{ki}_{lt}", tag="rw")
                        # partition: (lh in [2lt,2lt+2), lw in [0,64));
                        # rows r = 2*lh + ki, cols cc = 2*lw + kj
                        off = (4 * lt + ki) * 130 * 128
                        src = bass.AP(btp_d, off,
                                      [[2 * 130 * 128, 2], [2 * 128, 64],
                                       [128, 4], [1, 128]])
                        nc.sync.dma_start(rw, src)
                        rws.append(rw)
                    for kj in range(4):
                        for qh in range(2):
                            acc = psp.tile([128, 512], F32,
                                           name=f"acc_{ki}_{kj}_{qh}",
                                           tag="ps")
                            for lt in range(32):
                                nc.tensor.matmul(
                                    acc[:], rws[lt][:, kj, :],
                                    PT[:, lt, qh * 512:(qh + 1) * 512],
                                    start=(lt == 0), stop=(lt == 31))
                            osb = osp.tile([128, 512], F32,
                                           name=f"osb_{ki}_{kj}_{qh}",
                                           tag="osb")
                            nc.scalar.copy(osb[:], acc[:])
                            nc.sync.dma_start(
                                rec_d[ki * 4 + kj, :,
                                      qh * 512:(qh + 1) * 512], osb[:])
    nc.compile()
    return nc


def _get_nc(score_mode="f32r"):
    if score_mode not in _CACHE:
        _CACHE[score_mode] = _build(score_mode)
    return _CACHE[score_mode]


def _prep_inputs(f, b, mask):
    """Host-side layout prep + tiny per-patch scale computation."""
    f = np.ascontiguousarray(np.asarray(f, dtype=np.float32))
    b = np.ascontiguousarray(np.asarray(b, dtype=np.float32))
    mask = np.asarray(mask, dtype=np.float32)

    fs = f[:, :, ::2, ::2]                      # [2,128,64,64]
    bs = b[:, :, ::2, ::2]

    fs_pad = np.zeros((B, C, 66, 66), np.float32)
    fs_pad[:, :, 1:65, 1:65] = fs
    bs_pad = np.zeros((B, C, 66, 66), np.float32)
    bs_pad[:, :, 1:65, 1:65] = bs

    # per-patch L2 norms of 3x3xC windows of bs (matches reference order-ish)
    ssq = (bs * bs).sum(axis=1)                 # [2,64,64]
    ssq_pad = np.zeros((B, 66, 66), np.float32)
    ssq_pad[:, 1:65, 1:65] = ssq
    win = np.zeros((B, HF, WF), np.float32)
    for dp in range(3):
        for dq in range(3):
            win += ssq_pad[:, dp:dp + 64, dq:dq + 64]
    norm = np.sqrt(win).reshape(B, L)

    # patch validity from mask (batch 0 only, like the reference)
    ms = mask[0, 0, ::8, ::8]                   # [64,64]
    ms_pad = np.zeros((66, 66), np.float32)
    ms_pad[1:65, 1:65] = ms
    win9 = np.zeros((HF, WF), np.float32)
    for dp in range(3):
        for dq in range(3):
            win9 += ms_pad[dp:dp + 64, dq:dq + 64]
    mm = (win9 == 0.0).astype(np.float32).reshape(L)

    s = SCALE * mm[None, :] / np.maximum(norm, 1e-4)   # [2, L]

    # padded transposed b for raw-patch DMA, fp16
    btp = np.zeros((B, 130, 130, 128), np.float16)
    btp[:, 1:129, 1:129, :] = b.transpose(0, 2, 3, 1)

    mm_bc = np.ascontiguousarray(
        np.broadcast_to(mm[None, :], (128, L)), dtype=np.float32)

    in_maps = []
    cores = [(bi, cb) for bi in range(B) for cb in range(4)]
    for bi, cb in cores:
        fsq = np.empty((9, 128, QPC), np.float32)
        for t, (dp, dq) in enumerate(OFFS):
            blk = fs_pad[bi][:, 16 * cb + dp:16 * cb + dp + 16, dq:dq + 64]
            fsq[t] = blk.reshape(128, QPC)
        s_bc = np.ascontiguousarray(
            np.broadcast_to(s[bi][None, :], (128, L)), dtype=np.float32)
        in_maps.append({
            "fsq": fsq,
            "bsb": np.ascontiguousarray(bs_pad[bi]),
            "sbc": s_bc,
            "mmb": mm_bc,
            "btp": np.ascontiguousarray(btp[bi]),
        })
    extras = {"fs_pad": fs_pad, "bs_pad": bs_pad, "s": s}
    return in_maps, cores, extras


def _exact_rescore(bi, qlist, cand, fs_pad, bs_pad, s):
    """Exact fp64 scores for candidate patches of the given queries.

    qlist: [n] global query indices (h*64+w) in batch bi; cand: [n, 8].
    Returns [n, 8] scores matching the reference formula.
    """
    n = len(qlist)
    hq, wq = qlist // WF, qlist % WF
    hl, wl = cand // WF, cand % WF
    out = np.zeros((n, 8), np.float64)
    for dp in range(3):
        for dq in range(3):
            qv = fs_pad[bi][:, hq + dp, wq + dq].astype(np.float64)  # [C,n]
            kv = bs_pad[bi][:, hl + dp, wl + dq].astype(np.float64)  # [C,n,8]
            out += np.einsum("cn,cnk->nk", qv, kv)
    return out * s[bi][cand].astype(np.float64)


def kernel(f, b, mask):
    in_maps, cores, ex = _prep_inputs(f, b, mask)
    nc = _get_nc()
    res = run_bass_kernel_spmd(nc, in_maps, core_ids=list(range(8)))

    y_pad = np.zeros((B, C, 130, 130), np.float32)
    offsets = np.zeros((B, 2, HF, WF), np.int32)
    h_ref = np.arange(HF, dtype=np.int32)[:, None]
    w_ref = np.arange(WF, dtype=np.int32)[None, :]

    for (bi, cb), r in zip(cores, res.results):
        rec = r["recon"]                        # [16,128,1024]
        for ki in range(4):
            for kj in range(4):
                v = rec[ki * 4 + kj].reshape(128, 16, 64)
                y_pad[bi][:, 32 * cb + ki:32 * cb + ki + 32:2,
                          kj:kj + 128:2] += v
        cand = r["idxo"].reshape(QPC, 8).astype(np.int64)   # [1024, 8]
        tops = r["tops"].reshape(QPC, 8).astype(np.float64)
        best = cand[:, 0].copy()
        # device scores are ~tf32; exactly rescore queries whose top-2
        # probabilities are within the noise band (plus paranoia cases)
        amb = (tops[:, 1] >= tops[:, 0] * 0.72) | (tops[:, 0] <= 0.0)
        if amb.any():
            qi = np.nonzero(amb)[0]
            qglob = 16 * cb * WF + qi                       # h*64+w global
            sc = _exact_rescore(bi, qglob, cand[qi],
                                ex["fs_pad"], ex["bs_pad"], ex["s"])
            # argmax with first-occurrence (lowest l) tie-break
            m = sc.max(axis=1, keepdims=True)
            cpick = np.where(sc >= m, cand[qi], np.int64(L)).min(axis=1)
            best[qi] = cpick
        idx = best.reshape(16, 64).astype(np.int32)         # [h_local, w]
        hs = slice(16 * cb, 16 * cb + 16)
        offsets[bi, 0, hs, :] = idx // WF - h_ref[hs, :]
        offsets[bi, 1, hs, :] = idx % WF - w_ref

    y = y_pad[:, :, 1:129, 1:129]
    return y, offsets
